# revision 2
# baseline (speedup 1.0000x reference)
"""Trainium2 Bass kernel for nn_L2MLoRAqkv (MoE-routed LoRA QKV projection).

Math (per batch b, expert i = idx[b,0]):
    qkv = x @ W.T + bias
    qkv[:, :D]  += (x @ A_q[i]) @ B_q[i] * SCALE
    qkv[:, -D:] += (x @ A_v[i]) @ B_v[i] * SCALE

Strategy: data-parallel over the batch dim (1 batch per NeuronCore, 8 cores).
On the host we gather each batch's expert and fold the rank-8 LoRA update
into the (transposed) projection weight in float64, so the device kernel is
a single dense GEMM per core:
    Y[4096, 3072] = X[4096, 1024] @ W_eff[1024, 3072] + bias

PE floor is 1536 matmuls x 216ns = 331.8us; everything else is about keeping
the PE gapless and the HAM clock gate warm:
 - n-major "column-pass" schedule: pass n walks all 32 token tiles (8 chunks
   x 4 tiles, k-outer inside a 4-tile super-group).  New-byte DMA demand is
   flat ~148GB/s (x chunk pieces during pass 0, one w n-slice per pass start)
   instead of the 590GB/s burst a token-major schedule needs for its first
   t_block -- which is what stalled the PE (and re-throttled the HAM clock
   gate) in the previous version.
 - Inputs are host-packed so every DMA piece is a fully contiguous 128KB
   read: w is n-block-major [n][k][128][512], x is chunk-major
   [c][k][128][512].  Loads alternate across both HWDGE rings in exactly
   the order the PE consumes them; stores are emitted after all loads so
   ring FIFO order never lets a store's drain-wait block a load.
 - The HAM clock gate (1.2GHz cold) flips to 2.4GHz only after one fully
   busy free-running 3413ns window, and re-throttles after an idle window.
   gpsimd memsets the ones tile as soon as its queue is up (~6.3us) and
   ~26 K=1 warm-up matmuls + the 6 bias-replication matmuls keep the PE
   gapless until the first real data lands (~10.5us).
 - bias ships as one 12KB row and is replicated across partitions on-chip
   via ones[1,128].T @ bias_row matmuls (2 dedicated PSUM banks; the other
   6 banks rotate the accumulation groups).
 - Tail: the last super-group runs k-inner per group so drains/stores of
   its first 3 groups overlap the remaining matmuls, and the final group's
   drain+store is split in halves on both rings.
"""

import os
import sys

import numpy as np

for _p in ("/opt/trn_rl_repo",):
    if _p not in sys.path and os.path.isdir(_p):
        sys.path.insert(0, _p)

B = 8          # batches == cores
T = 4096       # tokens per batch
D = 1024       # model dim (contraction K)
N3 = 3072      # qkv output dim
P = 128        # SBUF partitions
NT = 512       # n-tile (one fp32 PSUM bank)
KT = D // P        # 8 k-tiles
NN = N3 // NT      # 6 n-blocks
CH = T // NT       # 8 token chunks
JJ = NT // P       # 4 token tiles per chunk
WARM = 26          # ones warm-ups before the bias matmuls
SCALE = 8.0 / 8.0

_NC_CACHE = {}


def _build():
    import concourse.tile as tile
    from concourse import bacc, mybir

    bf16 = mybir.dt.bfloat16
    f32 = mybir.dt.float32

    nc = bacc.Bacc(
        "TRN2",
        target_bir_lowering=False,
        debug=False,
        enable_asserts=False,
        num_devices=B,
    )
    wq = nc.dram_tensor("wq", [NN * KT * P, NT], bf16, kind="ExternalInput").ap()
    xq = nc.dram_tensor("xq", [CH * KT * P, NT], bf16, kind="ExternalInput").ap()
    biasr = nc.dram_tensor("biasr", [1, N3], bf16, kind="ExternalInput").ap()
    y = nc.dram_tensor("y", [T, N3], bf16, kind="ExternalOutput").ap()

    with tile.TileContext(nc) as tc:
        with tc.tile_pool(name="const", bufs=1) as const_pool, \
             tc.tile_pool(name="outp", bufs=8) as out_pool, \
             tc.tile_pool(name="ps", bufs=1, space="PSUM") as psum_pool:

            # w_sb col (n*KT+k)*NT: w[k, n*NT:(n+1)*NT]
            # x_sb col (c*KT+k)*NT + j*P: xT[k, (c*JJ+j)*P ...]
            w_sb = const_pool.tile([P, NN * KT * NT], bf16)
            x_sb = const_pool.tile([P, CH * KT * NT], bf16)
            bias_sb = const_pool.tile([P, N3], f32)
            bias_row = const_pool.tile([1, N3], bf16)
            ones_sb = const_pool.tile([1, P], bf16)

            # ones for warm-ups / bias replication: gpsimd's queue is up
            # earliest and is otherwise idle.
            nc.gpsimd.memset(ones_sb[:], 1.0)

            # ---- loads, in exact consumption order, alternating rings ----
            ring = [nc.scalar, nc.sync]
            rn = [0]

            def ld_piece(sb, col, dram, row):
                eng = ring[rn[0] % 2]
                rn[0] += 1
                eng.dma_start(sb[:, col : col + NT],
                              dram[row : row + P, :])

            def ld_w(n, k):
                ld_piece(w_sb, (n * KT + k) * NT, wq, (n * KT + k) * P)

            def ld_x(c, k):
                ld_piece(x_sb, (c * KT + k) * NT, xq, (c * KT + k) * P)

            nc.scalar.dma_start(bias_row[:], biasr[:])
            # head lockstep: the first super-group covers chunks 0+1 at n=0,
            # consuming per k-step exactly one [w(0,k), x(0,k), x(1,k)] trio.
            for k in range(KT):
                ld_w(0, k)
                ld_x(0, k)
                ld_x(1, k)
            # rest of x (pass-0 fuel), with the pass-1..5 w slices behind.
            for c in range(2, CH):
                for k in range(KT):
                    ld_x(c, k)
            for n in range(1, NN):
                for k in range(KT):
                    ld_w(n, k)

            # ---- PE warm-up: keep the HAM activity window gapless from
            # queue-up until real data lands.
            wu = psum_pool.tile([P, NT], f32, tag="psb", bufs=2, name="wu")
            for _ in range(WARM):
                nc.tensor.matmul(
                    wu[:, 0:P], lhsT=ones_sb[:], rhs=ones_sb[:],
                    start=True, stop=True,
                )
            # bias replication doubles as warm-up (bias_row lands ~8.8us).
            for n in range(NN):
                psb = psum_pool.tile([P, NT], f32, tag="psb", bufs=2, name="psb")
                nc.tensor.matmul(
                    psb[:],
                    lhsT=ones_sb[:],
                    rhs=bias_row[:, n * NT : (n + 1) * NT],
                    start=True,
                    stop=True,
                )
                nc.vector.tensor_copy(bias_sb[:, n * NT : (n + 1) * NT], psb[:])

            # ---- main schedule ----
            def w_ap(n, k):
                base = (n * KT + k) * NT
                return w_sb[:, base : base + NT]

            def x_ap(c, k, j):
                base = (c * KT + k) * NT + j * P
                return x_sb[:, base : base + P]

            def mm(ps, c, j, n, k):
                nc.tensor.matmul(
                    ps[:],
                    lhsT=x_ap(c, k, j),
                    rhs=w_ap(n, k),
                    start=(k == 0),
                    stop=(k == KT - 1),
                )

            stores = []
            gctr = [0]

            def drain_store(ps, c, j, n, split=False):
                ob = out_pool.tile([P, NT], bf16, tag="ob", name="ob")
                t = c * JJ + j
                parts = 2 if split else 1
                h = NT // parts
                for i in range(parts):
                    sl = slice(i * h, (i + 1) * h)
                    nc.vector.tensor_add(
                        ob[:, sl], ps[:, sl],
                        bias_sb[:, n * NT + i * h : n * NT + (i + 1) * h],
                    )
                    eng = ring[gctr[0] % 2]
                    gctr[0] += 1
                    eng.dma_start(
                        y[t * P : (t + 1) * P, n * NT + i * h : n * NT + (i + 1) * h],
                        ob[:, sl],
                    )

            def ps_tile():
                return psum_pool.tile([P, NT], f32, tag="ps", bufs=6, name="ps")

            def supergroup(n, groups, k_outer=True, split_last=False):
                pss = [ps_tile() for _ in groups]
                if k_outer:
                    for k in range(KT):
                        for g, (c, j) in enumerate(groups):
                            mm(pss[g], c, j, n, k)
                    for g, (c, j) in enumerate(groups):
                        drain_store(pss[g], c, j, n)
                else:
                    for g, (c, j) in enumerate(groups):
                        for k in range(KT):
                            mm(pss[g], c, j, n, k)
                        drain_store(pss[g], c, j, n,
                                    split=(split_last and g == len(groups) - 1))

            # pass 0: head super-group spans chunks 0+1 (6 groups = 6 banks),
            # then the two leftover chunk-1 tiles, then chunks 2..7.
            supergroup(0, [(0, 0), (0, 1), (0, 2), (0, 3), (1, 0), (1, 1)])
            supergroup(0, [(1, 2), (1, 3)])
            for c in range(2, CH):
                supergroup(0, [(c, j) for j in range(JJ)])
            # passes 1..5
            for n in range(1, NN):
                for c in range(CH):
                    last = (n == NN - 1 and c == CH - 1)
                    supergroup(n, [(c, j) for j in range(JJ)],
                               k_outer=not last, split_last=last)
    nc.compile()
    return nc


def _get_nc():
    if "v2" not in _NC_CACHE:
        _NC_CACHE["v2"] = _build()
    return _NC_CACHE["v2"]


def _prep_in_maps(inputs):
    import ml_dtypes

    bf16 = ml_dtypes.bfloat16

    x = np.asarray(inputs["x"], dtype=np.float32)
    weight = np.asarray(inputs["weight"], dtype=np.float32)
    bias = np.asarray(inputs["bias"], dtype=np.float32)
    aq = np.asarray(inputs["A_q_pool"], dtype=np.float32)
    bq = np.asarray(inputs["B_q_pool"], dtype=np.float32)
    av = np.asarray(inputs["A_v_pool"], dtype=np.float32)
    bv = np.asarray(inputs["B_v_pool"], dtype=np.float32)
    idx = np.asarray(inputs["idx"]).reshape(B, -1)[:, 0].astype(np.int64)

    wt64 = weight.T.astype(np.float64)  # [D, N3]
    biasr = np.ascontiguousarray(bias.reshape(1, N3)).astype(bf16)
    xts = x.transpose(0, 2, 1)  # [B, D, T] strided view

    in_maps = []
    for b in range(B):
        i = int(idx[b])
        weff = wt64.copy()
        weff[:, :D] += SCALE * (aq[i].astype(np.float64) @ bq[i].astype(np.float64))
        weff[:, N3 - D:] += SCALE * (av[i].astype(np.float64) @ bv[i].astype(np.float64))
        weffb = weff.astype(np.float32).astype(bf16)        # [D, N3]
        xtb = np.ascontiguousarray(xts[b]).astype(bf16)     # [D, T]
        # w: n-block-major [n][k][128][512]
        wqb = np.ascontiguousarray(
            weffb.reshape(KT, P, NN, NT).transpose(2, 0, 1, 3)
        ).reshape(NN * KT * P, NT)
        # x: chunk-major [c][k][128][512]
        xqb = np.ascontiguousarray(
            xtb.reshape(KT, P, CH, NT).transpose(2, 0, 1, 3)
        ).reshape(CH * KT * P, NT)
        in_maps.append({
            "wq": np.ascontiguousarray(wqb),
            "xq": np.ascontiguousarray(xqb),
            "biasr": biasr,
        })
    return in_maps


def _run(in_maps, trace=False, **kwargs):
    from concourse.bass_utils import run_bass_kernel_spmd

    nc = _get_nc()
    return run_bass_kernel_spmd(
        nc, in_maps, core_ids=list(range(B)), trace=trace, **kwargs
    )


def kernel(**inputs):
    res = _run(_prep_in_maps(inputs), trace=False)
    return np.stack(
        [np.asarray(r["y"], dtype=np.float32) for r in res.results], axis=0
    )


# revision 3
# speedup vs baseline: 1.2496x; 1.2496x over previous
"""Trainium2 Bass kernel for nn_L2MLoRAqkv (MoE-routed LoRA QKV projection).

Math (per batch b, expert i = idx[b,0]):
    qkv = x @ W.T + bias
    qkv[:, :D]  += (x @ A_q[i]) @ B_q[i] * SCALE
    qkv[:, -D:] += (x @ A_v[i]) @ B_v[i] * SCALE

Strategy: data-parallel over the batch dim (1 batch per NeuronCore, 8 cores).
On the host we gather each batch's expert and fold the rank-8 LoRA update
into the (transposed) projection weight in float64, so the device kernel is
a single dense GEMM per core:
    Y[4096, 3072] = X[4096, 1024] @ W_eff[1024, 3072] + bias

PE floor is 1536 matmuls x 216ns = 331.8us.  Design notes (from traces):
 - LDWEIGHTS issue is only hidden when >=2 consecutive matmuls share the
   stationary operand (a width-1 schedule costs +43ns/mm).  So: phase A
   covers n0+n1 for every token tile (width-2 reuse of each x tile) while
   x streams in chunk by chunk; phase B covers n2..n5 (width-4) with
   everything resident.  Phase A's new-byte demand is ~75GB/s, phase B's
   is zero -- no DMA stall, so the HAM clock gate never re-throttles.
 - Each dma_start costs ~690ns of HWDGE ring trigger time regardless of
   size, so bulk data moves as 1MB single DMAs (8KB contiguous partition
   lines).  Only the startup-critical head data (w n0/n1 + x chunk 0,
   consumed k-step by k-step before the rings are warm) is split into
   per-k 128KB pieces, ring-alternated in exact consumption order.
 - The HAM clock gate (1.2GHz cold) flips to 2.4GHz only after one fully
   busy free-running 3413ns window, and re-throttles after any idle
   window.  gpsimd memsets the ones tile as soon as its queue is up
   (~6.3us); ~12 K=1 warm-ups plus the 6 bias-replication matmuls keep
   the PE gapless until the first head pieces land (~10us).
 - bias ships as one 12KB row and is replicated across partitions on-chip
   via ones[1,128].T @ bias_row matmuls into 2 PSUM banks during warm-up;
   the other groups rotate freely through all 8 banks (phase A head uses
   8, steady-state super-groups 4 -> fully double-buffered, no gaps).
 - Stores coalesce per token tile ([128,1024] in phase A, [128,2048] in
   phase B; 2-4KB lines) and are emitted after all loads so ring FIFO
   order can never block a load on an unfinished drain.  The final block
   runs n-major so its drains/stores overlap the remaining matmuls, and
   the very last 512-col group drains and stores in two halves.
"""

import os
import sys

import numpy as np

for _p in ("/opt/trn_rl_repo",):
    if _p not in sys.path and os.path.isdir(_p):
        sys.path.insert(0, _p)

B = 8          # batches == cores
T = 4096       # tokens per batch
D = 1024       # model dim (contraction K)
N3 = 3072      # qkv output dim
P = 128        # SBUF partitions
NT = 512       # n-tile (one fp32 PSUM bank)
KT = D // P        # 8 k-tiles
NN = N3 // NT      # 6 n-blocks
CH = T // NT       # 8 token chunks
JJ = NT // P       # 4 token tiles per chunk
NA = 2             # n-blocks done in phase A (width-2 lhsT reuse)
NB = NN - NA       # n-blocks done in phase B (width-4 lhsT reuse)
WARM = 12          # ones warm-ups before the bias matmuls
SCALE = 8.0 / 8.0

_NC_CACHE = {}


def _build():
    import concourse.tile as tile
    from concourse import bacc, mybir

    bf16 = mybir.dt.bfloat16
    f32 = mybir.dt.float32

    nc = bacc.Bacc(
        "TRN2",
        target_bir_lowering=False,
        debug=False,
        enable_asserts=False,
        num_devices=B,
    )
    # wq rows: n*P + p, cols: k*NT + c  (one 1MB contiguous n-slice per n)
    wq = nc.dram_tensor("wq", [NN * P, KT * NT], bf16, kind="ExternalInput").ap()
    # wh rows: (hh*KT+k)*P + p, cols: c  (head per-k pieces for n0/n1)
    wh = nc.dram_tensor("wh", [NA * KT * P, NT], bf16, kind="ExternalInput").ap()
    # xq rows: c*P + p, cols: k*NT + cc (one 1MB contiguous chunk per c)
    xq = nc.dram_tensor("xq", [CH * P, KT * NT], bf16, kind="ExternalInput").ap()
    # xh rows: k*P + p, cols: cc (head per-k pieces of chunk 0)
    xh = nc.dram_tensor("xh", [KT * P, NT], bf16, kind="ExternalInput").ap()
    biasr = nc.dram_tensor("biasr", [1, N3], bf16, kind="ExternalInput").ap()
    y = nc.dram_tensor("y", [T, N3], bf16, kind="ExternalOutput").ap()

    with tile.TileContext(nc) as tc:
        with tc.tile_pool(name="const", bufs=1) as const_pool, \
             tc.tile_pool(name="outp", bufs=1) as out_pool, \
             tc.tile_pool(name="ps", bufs=1, space="PSUM") as psum_pool:

            # w_sb col (n*KT+k)*NT: w[k, n*NT:(n+1)*NT]
            # x_sb col (c*KT+k)*NT + j*P: xT[k, (c*JJ+j)*P ...]
            w_sb = const_pool.tile([P, NN * KT * NT], bf16)
            x_sb = const_pool.tile([P, CH * KT * NT], bf16)
            bias_sb = const_pool.tile([P, N3], f32)
            bias_row = const_pool.tile([1, N3], bf16)
            ones_sb = const_pool.tile([1, P], bf16)

            # ones for warm-ups / bias replication: gpsimd's queue is up
            # earliest and is otherwise idle.
            nc.gpsimd.memset(ones_sb[:], 1.0)

            # ---- loads, in exact consumption order, alternating rings ----
            ring = [nc.scalar, nc.sync]
            rn = [0]

            def pick():
                eng = ring[rn[0] % 2]
                rn[0] += 1
                return eng

            nc.scalar.dma_start(bias_row[:], biasr[:])
            # head pieces: per k, [w(0,k), x(0,k), w(1,k)] in consumption
            # order (the head super-group eats one trio per k-step).
            for k in range(KT):
                pick().dma_start(
                    w_sb[:, k * NT : (k + 1) * NT],
                    wh[k * P : (k + 1) * P, :],
                )
                pick().dma_start(
                    x_sb[:, k * NT : (k + 1) * NT],
                    xh[k * P : (k + 1) * P, :],
                )
                pick().dma_start(
                    w_sb[:, (KT + k) * NT : (KT + k + 1) * NT],
                    wh[(KT + k) * P : (KT + k + 1) * P, :],
                )
            # bulk x chunks 1..7, then the phase-B w slices: 1MB single DMAs.
            for c in range(1, CH):
                pick().dma_start(
                    x_sb[:, c * KT * NT : (c + 1) * KT * NT],
                    xq[c * P : (c + 1) * P, :],
                )
            for n in range(NA, NN):
                pick().dma_start(
                    w_sb[:, n * KT * NT : (n + 1) * KT * NT],
                    wq[n * P : (n + 1) * P, :],
                )

            # ---- PE warm-up: keep the HAM activity window gapless from
            # queue-up until the first head pieces land.
            wu = psum_pool.tile([P, NT], f32, tag="ps", bufs=8, name="wu")
            for _ in range(WARM):
                nc.tensor.matmul(
                    wu[:, 0:P], lhsT=ones_sb[:], rhs=ones_sb[:],
                    start=True, stop=True,
                )
            # bias replication doubles as warm-up (bias_row lands ~8.8us).
            for n in range(NN):
                psb = psum_pool.tile([P, NT], f32, tag="ps", bufs=8, name="psb")
                nc.tensor.matmul(
                    psb[:],
                    lhsT=ones_sb[:],
                    rhs=bias_row[:, n * NT : (n + 1) * NT],
                    start=True,
                    stop=True,
                )
                nc.vector.tensor_copy(bias_sb[:, n * NT : (n + 1) * NT], psb[:])

            # ---- main schedule ----
            def w_ap(n, k):
                base = (n * KT + k) * NT
                return w_sb[:, base : base + NT]

            def x_ap(c, k, j):
                base = (c * KT + k) * NT + j * P
                return x_sb[:, base : base + P]

            def mm(ps, c, j, n, k):
                nc.tensor.matmul(
                    ps[:],
                    lhsT=x_ap(c, k, j),
                    rhs=w_ap(n, k),
                    start=(k == 0),
                    stop=(k == KT - 1),
                )

            def ps_tile():
                return psum_pool.tile([P, NT], f32, tag="ps", bufs=8, name="ps")

            gctr = [0]

            def store(dst_ap, src_ap):
                eng = ring[gctr[0] % 2]
                gctr[0] += 1
                eng.dma_start(dst_ap, src_ap)

            # Phase A: per chunk, super-groups of (tiles x n0..1), k-outer,
            # each x tile stationary across its NA consecutive matmuls.
            def sga(c, js):
                pss = {(j, n): ps_tile() for j in js for n in range(NA)}
                for k in range(KT):
                    for j in js:
                        for n in range(NA):
                            mm(pss[(j, n)], c, j, n, k)
                for j in js:
                    ob = out_pool.tile([P, NA * NT], bf16, tag="oba", bufs=8,
                                       name="ob")
                    for n in range(NA):
                        nc.vector.tensor_add(
                            ob[:, n * NT : (n + 1) * NT],
                            pss[(j, n)][:],
                            bias_sb[:, n * NT : (n + 1) * NT],
                        )
                    t = c * JJ + j
                    store(y[t * P : (t + 1) * P, 0 : NA * NT], ob[:])

            sga(0, [0, 1, 2, 3])         # head: 8 banks, one trio per k-step
            for c in range(1, CH):
                sga(c, [0, 1])
                sga(c, [2, 3])

            # Phase B: per token tile, n2..n5 k-outer / n-inner (width-4
            # stationary reuse), everything SBUF-resident.
            def blk(t):
                c, j = divmod(t, JJ)
                pss = [ps_tile() for _ in range(NB)]
                for k in range(KT):
                    for g in range(NB):
                        mm(pss[g], c, j, NA + g, k)
                ob = out_pool.tile([P, NB * NT], bf16, tag="obb", bufs=6,
                                   name="ob")
                for g in range(NB):
                    nc.vector.tensor_add(
                        ob[:, g * NT : (g + 1) * NT],
                        pss[g][:],
                        bias_sb[:, (NA + g) * NT : (NA + g + 1) * NT],
                    )
                store(y[t * P : (t + 1) * P, NA * NT : N3], ob[:])

            def blk_final(t):
                # n-major so drains/stores overlap remaining matmuls; the
                # last group drains and stores in halves on both rings.
                c, j = divmod(t, JJ)
                for g in range(NB):
                    ps = ps_tile()
                    for k in range(KT):
                        mm(ps, c, j, NA + g, k)
                    n = NA + g
                    ob = out_pool.tile([P, NT], bf16, tag="obf", bufs=2,
                                       name="ob")
                    parts = 2 if g == NB - 1 else 1
                    h = NT // parts
                    for i in range(parts):
                        sl = slice(i * h, (i + 1) * h)
                        nc.vector.tensor_add(
                            ob[:, sl], ps[:, sl],
                            bias_sb[:, n * NT + i * h : n * NT + (i + 1) * h],
                        )
                        store(
                            y[t * P : (t + 1) * P,
                              n * NT + i * h : n * NT + (i + 1) * h],
                            ob[:, sl],
                        )

            for t in range(T // P - 1):
                blk(t)
            blk_final(T // P - 1)
    nc.compile()
    return nc


def _get_nc():
    if "v3" not in _NC_CACHE:
        _NC_CACHE["v3"] = _build()
    return _NC_CACHE["v3"]


def _prep_in_maps(inputs):
    import ml_dtypes

    bf16 = ml_dtypes.bfloat16

    x = np.asarray(inputs["x"], dtype=np.float32)
    weight = np.asarray(inputs["weight"], dtype=np.float32)
    bias = np.asarray(inputs["bias"], dtype=np.float32)
    aq = np.asarray(inputs["A_q_pool"], dtype=np.float32)
    bq = np.asarray(inputs["B_q_pool"], dtype=np.float32)
    av = np.asarray(inputs["A_v_pool"], dtype=np.float32)
    bv = np.asarray(inputs["B_v_pool"], dtype=np.float32)
    idx = np.asarray(inputs["idx"]).reshape(B, -1)[:, 0].astype(np.int64)

    wt64 = weight.T.astype(np.float64)  # [D, N3]
    biasr = np.ascontiguousarray(bias.reshape(1, N3)).astype(bf16)
    xts = x.transpose(0, 2, 1)  # [B, D, T] strided view

    in_maps = []
    for b in range(B):
        i = int(idx[b])
        weff = wt64.copy()
        weff[:, :D] += SCALE * (aq[i].astype(np.float64) @ bq[i].astype(np.float64))
        weff[:, N3 - D:] += SCALE * (av[i].astype(np.float64) @ bv[i].astype(np.float64))
        weffb = weff.astype(np.float32).astype(bf16)        # [D, N3]
        xtb = np.ascontiguousarray(xts[b]).astype(bf16)     # [D, T]
        # bulk w: rows n*P+p, cols k*NT+c
        w4 = weffb.reshape(KT, P, NN, NT)
        wqb = np.ascontiguousarray(
            w4.transpose(2, 1, 0, 3).reshape(NN * P, KT * NT)
        )
        # head w: per-k pieces for n0..n1
        whb = np.ascontiguousarray(
            w4[:, :, :NA].transpose(2, 0, 1, 3).reshape(NA * KT * P, NT)
        )
        # bulk x: rows c*P+p, cols k*NT+cc
        x4 = xtb.reshape(KT, P, CH, NT)
        xqb = np.ascontiguousarray(
            x4.transpose(2, 1, 0, 3).reshape(CH * P, KT * NT)
        )
        # head x: per-k pieces of chunk 0
        xhb = np.ascontiguousarray(
            x4[:, :, 0].reshape(KT * P, NT)
        )
        in_maps.append({
            "wq": wqb,
            "wh": whb,
            "xq": xqb,
            "xh": xhb,
            "biasr": biasr,
        })
    return in_maps


def _run(in_maps, trace=False, **kwargs):
    from concourse.bass_utils import run_bass_kernel_spmd

    nc = _get_nc()
    return run_bass_kernel_spmd(
        nc, in_maps, core_ids=list(range(B)), trace=trace, **kwargs
    )


def kernel(**inputs):
    res = _run(_prep_in_maps(inputs), trace=False)
    return np.stack(
        [np.asarray(r["y"], dtype=np.float32) for r in res.results], axis=0
    )
